# revision 40
# baseline (speedup 1.0000x reference)
"""Multi-head attention (B=4, S=2048, D=1024, H=16, dk=dv=64) on 8 Trainium2
NeuronCores.

Sharding: core c handles batch b = c//2 and head-group g = c%2 (8 of 16 heads).

Host pre-processing: X^T (per batch) is transposed and cast to bf16 on the
host, so the device does no input transposes at all; weights are uploaded in
bf16. The v-projection bias and o-projection bias are linear post-terms
(wts += bv;  out += bv @ Wo + bo) and are applied on the host after the
partial-sum gather, so the device never touches them.

Per core (all matmuls in bf16, PSUM accumulation in fp32):
  - Q^T/K^T [512, 2048] projected with W as the stationary operand and X^T
    streaming; q/k biases folded into the PSUM->SBUF eviction
    (vector tensor_scalar).
  - V is projected directly into its natural [2048, 8x(64+1)] layout
    (stationary = X_v^T tiles), with a ones column per head so the PV matmul
    also produces softmax row-sums.
  - Per head-pair j and 512-wide query chunk: scores^T = K Q^T via K=64
    row-tiled matmuls, exp on the scalar engine straight out of PSUM (mask as
    per-partition bias, 1/sqrt(dk) as scale) -> bf16, PV accumulated over the
    16 key tiles.
  - Context (+rowsum row) is transposed back to natural (bf16, 65-row PE
    transposes), normalized with reciprocal rowsums into bf16 `weights`
    (DMA'd out; host casts to f32), transposed once more to head-dim-major
    for the o_proj stationary operand.
  - Post-processing chunks are deferred and interleaved into later attention
    iterations so they fill PE gaps left by the exp dependency chain.
Host: slices inputs per core, sums the o_proj partials of each core pair,
adds bv@Wo+bo, and concatenates the weights halves (+bv).
"""
import sys

for _p in ("/opt/trn_rl_repo", "/root/.axon_site/_ro/trn_rl_repo"):
    if _p not in sys.path:
        sys.path.insert(0, _p)

import numpy as np
import ml_dtypes
import concourse.bass as bass
import concourse.bacc as bacc
import concourse.tile as tile
from concourse import mybir
from concourse.masks import make_identity
from concourse.bass_utils import run_bass_kernel_spmd

F32 = mybir.dt.float32
BF16 = mybir.dt.bfloat16
EXP = mybir.ActivationFunctionType.Exp
ADD = mybir.AluOpType.add
MULT = mybir.AluOpType.mult

B, S, D = 4, 2048, 1024
H, DK, DV = 16, 64, 64
NCORES = 8
HC = H // 2          # heads per core
HDK = HC * DK        # 512 head dims per core
SQC = 512            # query-chunk width


def build_program(nc: bass.Bass, s=S, d=D, hc=HC):
    hdk = hc * DK
    ck_n = hdk // 128        # 128-wide head-dim tiles (= head pairs)
    dt_n = d // 128          # D contraction tiles
    skt_n = s // 128         # key tiles
    sq_n = s // SQC          # query chunks
    ab = 1024                # phase-A S-block width
    abn = s // ab
    zn = SQC // 128

    xqT = nc.dram_tensor("xqT", [d, s], BF16, kind="ExternalInput")
    xkT = nc.dram_tensor("xkT", [d, s], BF16, kind="ExternalInput")
    xvT = nc.dram_tensor("xvT", [d, s], BF16, kind="ExternalInput")
    wq = nc.dram_tensor("wq", [d, hdk], BF16, kind="ExternalInput")
    wk = nc.dram_tensor("wk", [d, hdk], BF16, kind="ExternalInput")
    wv = nc.dram_tensor("wv", [d, hdk], BF16, kind="ExternalInput")
    bq = nc.dram_tensor("bq", [ck_n, 128, 1], F32, kind="ExternalInput")
    bk = nc.dram_tensor("bk", [ck_n, 128, 1], F32, kind="ExternalInput")
    wo = nc.dram_tensor("wo", [hdk, d], BF16, kind="ExternalInput")
    msk = nc.dram_tensor("msk", [skt_n, 128, 1], F32, kind="ExternalInput")

    out_p = nc.dram_tensor("out_p", [s, d], F32, kind="ExternalOutput")
    wts_p = nc.dram_tensor("wts_p", [s, hdk], BF16, kind="ExternalOutput")

    # weights DMA view: rows (q, z, p), cols (pair j, head m, dv)
    wts_v = wts_p.rearrange(
        "(q z p) (j m e) -> q j p m z e", z=zn, p=128, m=2, e=DV
    )

    with tile.TileContext(nc) as tc, \
            tc.tile_pool(name="consts", bufs=1) as consts, \
            tc.tile_pool(name="persist", bufs=1) as persist:
        ident = consts.tile([128, 128], F32, name="ident")
        make_identity(nc, ident)
        identb = consts.tile([128, 128], BF16, name="identb")
        nc.vector.tensor_copy(identb, ident)
        msk_sb = consts.tile([128, skt_n], F32, name="msk_sb")
        nc.sync.dma_start(out=msk_sb, in_=msk.rearrange("t p one -> p (t one)"))
        bias_t = {}
        for nm, src in (("q", bq), ("k", bk)):
            bt = consts.tile([128, ck_n], F32, name=f"b{nm}_t")
            nc.sync.dma_start(out=bt, in_=src.rearrange("t p one -> p (t one)"))
            bias_t[nm] = bt

        qT = persist.tile([128, ck_n, s], BF16, name="qT")
        kT = persist.tile([128, ck_n, s], BF16, name="kT")
        # DV+2 lanes per head: [64 dv | ones (rowsum col) | zeros (pad so the
        # 66-wide bf16 context keeps PSUM accesses 4-byte aligned)]
        vtn = persist.tile([128, skt_n, hc, DV + 2], BF16, name="vtn")
        ones_th = consts.tile([128, skt_n * hc], BF16, name="ones_th")
        nc.gpsimd.memset(ones_th, 1.0)
        zs_th = consts.tile([128, skt_n * hc], BF16, name="zs_th")
        nc.gpsimd.memset(zs_th, 0.0)
        nc.vector.tensor_copy(
            vtn[:, :, :, DV : DV + 1],
            ones_th.rearrange("p (t h one) -> p t h one", t=skt_n, one=1),
        )
        nc.vector.tensor_copy(
            vtn[:, :, :, DV + 1 : DV + 2],
            zs_th.rearrange("p (t h one) -> p t h one", t=skt_n, one=1),
        )
        wo_sb = persist.tile([128, ck_n, d], BF16, name="wo_sb")

        # ---------------- Phase A: projections (no transposes) --------------
        # Emission order: K (all), Q (first S-block), pre-computed scores+exp
        # for (q0, j0) so the scalar engine starts ~60us earlier, Q (second
        # block), V. The sc/ep pools are opened before phase A so the
        # pre-scores can use them.
        pre_eps = {}
        with (
            tc.tile_pool(name="ep", bufs=38) as ep_pool,
            tc.tile_pool(name="sc_ps", bufs=2, space="PSUM") as sc_ps,
        ):
            with (
                tc.tile_pool(name="wz", bufs=2) as wz_pool,
                tc.tile_pool(name="xT", bufs=2) as xT_pool,
                tc.tile_pool(name="pa_ps", bufs=2, space="PSUM") as pa_ps,
            ):
                def load_w(wz):
                    w_sb = wz_pool.tile([128, dt_n, hdk], BF16, name="w_sb")
                    for t in range(dt_n):
                        nc.sync.dma_start(
                            out=w_sb[:, t], in_=wz[t * 128 : (t + 1) * 128, :]
                        )
                    return w_sb

                def load_xT(xzT, blk):
                    # all X^T loads on the SP queue: the DGE is bandwidth-
                    # bound (~130GB/s/core), so spreading across HWDGE queues
                    # gains nothing and costs Activation-queue issue time
                    xT_sb = xT_pool.tile([128, dt_n, ab], BF16, name="xT_sb")
                    for t in range(dt_n):
                        nc.sync.dma_start(
                            out=xT_sb[:, t],
                            in_=xzT[t * 128 : (t + 1) * 128, blk * ab : (blk + 1) * ab],
                        )
                    return xT_sb

                def emit_qk_blk(w_sb, xT_sb, outT, bnm, blk):
                    for ck in range(ck_n):
                        pp = pa_ps.tile([128, ab], F32, name="proj_pp")
                        for dt_ in range(dt_n):
                            lhsT = w_sb[:, dt_, ck * 128 : (ck + 1) * 128]
                            for h2 in range(ab // 512):
                                nc.tensor.matmul(
                                    pp[:, h2 * 512 : (h2 + 1) * 512],
                                    lhsT,
                                    xT_sb[:, dt_, h2 * 512 : (h2 + 1) * 512],
                                    start=(dt_ == 0),
                                    stop=(dt_ == dt_n - 1),
                                )
                        nc.vector.tensor_scalar(
                            out=outT[:, ck, blk * ab : (blk + 1) * ab],
                            in0=pp,
                            scalar1=bias_t[bnm][:, ck : ck + 1],
                            scalar2=None,
                            op0=ADD,
                        )

                def emit_scores_exp(q, j):
                    q0 = q * SQC
                    sc = sc_ps.tile([128, 2 * SQC], F32, name="sc_t")
                    eps = []
                    for t in range(skt_n):
                        sc = sc_ps.tile([128, 2 * SQC], F32, name="sc_t")
                        for m in range(2):
                            lo, hi = m * 64, (m + 1) * 64
                            nc.tensor.matmul(
                                sc[:, m * SQC : (m + 1) * SQC],
                                kT[lo:hi, j, t * 128 : (t + 1) * 128],
                                qT[lo:hi, j, q0 : q0 + SQC],
                                start=True, stop=True,
                                tile_position=(m * 64, 0),
                            )
                        ep = ep_pool.tile([128, 2 * SQC], BF16, name="ep_t")
                        nc.scalar.activation(
                            ep, sc, EXP, bias=msk_sb[:, t : t + 1], scale=0.125
                        )
                        eps.append(ep)
                    return eps

                # interleave the first weight/x loads so the first matmul
                # can start after ~2 DMAs instead of ~9
                wk_sb = wz_pool.tile([128, dt_n, hdk], BF16, name="w_sb")
                xk0_sb = xT_pool.tile([128, dt_n, ab], BF16, name="xT_sb")
                for t in range(dt_n):
                    nc.sync.dma_start(out=wk_sb[:, t], in_=wk[t * 128 : (t + 1) * 128, :])
                    nc.sync.dma_start(
                        out=xk0_sb[:, t], in_=xkT[t * 128 : (t + 1) * 128, 0:ab]
                    )
                emit_qk_blk(wk_sb, xk0_sb, kT, "k", 0)
                emit_qk_blk(wk_sb, load_xT(xkT, 1), kT, "k", 1)
                wq_sb = persist.tile([128, dt_n, hdk], BF16, name="wq_sb")
                for t in range(dt_n):
                    nc.sync.dma_start(out=wq_sb[:, t], in_=wq[t * 128 : (t + 1) * 128, :])
                emit_qk_blk(wq_sb, load_xT(xqT, 0), qT, "q", 0)
                pre_eps = {0: emit_scores_exp(0, 0), 1: emit_scores_exp(0, 1)}
                emit_qk_blk(wq_sb, load_xT(xqT, 1), qT, "q", 1)

                # V natural: stationary = X_v^T tile, moving = W_v (no bias —
                # the host folds bv into wts/out afterwards).
                wv_sb = load_w(wv)
                for blk in range(abn):
                    xT_sb = load_xT(xvT, blk)
                    for sub in range(ab // 128):
                        st = blk * (ab // 128) + sub
                        pv = pa_ps.tile([128, ab], F32, name="proj_pp")[:, 0:hdk]
                        for dt_ in range(dt_n):
                            nc.tensor.matmul(
                                pv,
                                xT_sb[:, dt_, sub * 128 : (sub + 1) * 128],
                                wv_sb[:, dt_],
                                start=(dt_ == 0),
                                stop=(dt_ == dt_n - 1),
                            )
                        nc.scalar.copy(
                            vtn[:, st, :, 0:DV],
                            pv.rearrange("p (h e) -> p h e", h=hc),
                        )
                for t in range(ck_n):
                    nc.sync.dma_start(out=wo_sb[:, t], in_=wo[t * 128 : (t + 1) * 128, :])

            # ---------------- Phase B: attention + o_proj -------------------
            with (
                tc.tile_pool(name="ctxu", bufs=3) as ctxu_pool,
                tc.tile_pool(name="wtsT", bufs=2) as wtsT_pool,
                tc.tile_pool(name="wnat", bufs=3) as wnat_pool,
                tc.tile_pool(name="rcp", bufs=3) as rcp_pool,
                tc.tile_pool(name="outsb", bufs=3) as outsb_pool,
                tc.tile_pool(name="ctx_ps", bufs=2, space="PSUM") as ctx_ps,
                tc.tile_pool(name="aux_ps", bufs=2, space="PSUM") as aux_ps,
            ):
                # Deferred post-processing (normalization / re-transposes /
                # o_proj), interleaved into later iterations' attention loops.
                pending = []


                def weights_chunk(q, j, m, ctxu, wnat, rc):
                    # the (q3, j3) chunk runs after the last exp — its DMA can
                    # use the idle Activation HWDGE queue instead of Sync
                    deng = nc.scalar if (q == sq_n - 1 and j == ck_n - 1) else nc.sync
                    def emit():
                        nat = aux_ps.tile([128, zn, DV + 2], BF16, name="aux")
                        for zz in range(zn):
                            nc.tensor.transpose(
                                nat[:, zz],
                                ctxu[:, m * SQC + zz * 128 : m * SQC + (zz + 1) * 128],
                                identb[0 : DV + 2, 0 : DV + 2],
                            )
                        nc.vector.reciprocal(rc[:, m], nat[:, :, DV : DV + 1])
                        for zz in range(zn):
                            nc.vector.tensor_scalar(
                                out=wnat[:, zz, m],
                                in0=nat[:, zz, 0:DV],
                                scalar1=rc[:, m, zz],
                                scalar2=None,
                                op0=MULT,
                            )
                        deng.dma_start(out=wts_v[q, j, :, m], in_=wnat[:, :, m, :])
                    return emit

                def wtsT_chunk(q, j, zz, wnat, wtsT_sb):
                    def emit():
                        wtp = aux_ps.tile([128, 128], BF16, name="aux")
                        nc.tensor.transpose(
                            wtp, wnat[:, zz].rearrange("p a b -> p (a b)"), identb
                        )
                        nc.vector.tensor_copy(
                            wtsT_sb[:, j, zz * 128 : (zz + 1) * 128], wtp
                        )
                    return emit

                def oproj_chunk(q, zz, h2, wtsT_sb, out_sb):
                    deng = nc.scalar if q == sq_n - 1 else nc.sync
                    def emit():
                        op = aux_ps.tile([128, 512], F32, name="aux")
                        for dt_ in range(ck_n):
                            nc.tensor.matmul(
                                op,
                                wtsT_sb[:, dt_, zz * 128 : (zz + 1) * 128],
                                wo_sb[:, dt_, h2 * 512 : (h2 + 1) * 512],
                                start=(dt_ == 0), stop=(dt_ == ck_n - 1),
                            )
                        nc.vector.tensor_copy(out_sb[:, h2 * 512 : (h2 + 1) * 512], op)
                        r0 = q * SQC + zz * 128
                        deng.dma_start(
                            out=out_p[r0 : r0 + 128, h2 * 512 : (h2 + 1) * 512],
                            in_=out_sb[:, h2 * 512 : (h2 + 1) * 512],
                        )
                    return emit

                for q in range(sq_n):
                    q0 = q * SQC
                    wtsT_sb = wtsT_pool.tile([128, ck_n, SQC], BF16, name="wtsT_sb")
                    for j in range(ck_n):
                        ctxA = ctx_ps.tile([DV + 2, SQC], F32, name="ctx_t")
                        ctxB = ctx_ps.tile([DV + 2, SQC], F32, name="ctx_t")

                        def emit_pv(ep, t):
                            nc.tensor.matmul(
                                ctxA, vtn[:, t, 2 * j], ep[:, 0:SQC],
                                start=(t == 0), stop=(t == skt_n - 1),
                            )
                            nc.tensor.matmul(
                                ctxB, vtn[:, t, 2 * j + 1], ep[:, SQC : 2 * SQC],
                                start=(t == 0), stop=(t == skt_n - 1),
                            )

                        # PV is pipelined one key-tile behind scores/exp so the
                        # PE never waits on the exp of the tile it just scored.
                        prev = None
                        for t in range(skt_n):
                            if q == 0 and j in pre_eps:
                                ep = pre_eps[j][t]
                            else:
                                sc = sc_ps.tile([128, 2 * SQC], F32, name="sc_t")
                                for m in range(2):
                                    lo, hi = m * 64, (m + 1) * 64
                                    nc.tensor.matmul(
                                        sc[:, m * SQC : (m + 1) * SQC],
                                        kT[lo:hi, j, t * 128 : (t + 1) * 128],
                                        qT[lo:hi, j, q0 : q0 + SQC],
                                        start=True, stop=True,
                                        tile_position=(m * 64, 0),
                                    )
                                ep = ep_pool.tile([128, 2 * SQC], BF16, name="ep_t")
                                nc.scalar.activation(
                                    ep, sc, EXP, bias=msk_sb[:, t : t + 1], scale=0.125
                                )
                            # deferred chunk runs between scores(t) and
                            # PV(t-1): the scores pair alone (~350ns) is too
                            # short to cover exp(t-1) (~1.1us)
                            if pending:
                                pending.pop(0)()
                            if prev is not None:
                                emit_pv(*prev)
                            prev = (ep, t)
                        emit_pv(*prev)
                        ctxu = ctxu_pool.tile([DV + 2, 2 * SQC], BF16, name="ctxu_t")
                        nc.vector.tensor_copy(ctxu[:, 0:SQC], ctxA)
                        nc.vector.tensor_copy(ctxu[:, SQC : 2 * SQC], ctxB)
                        wnat = wnat_pool.tile([128, zn, 2, DV], BF16, name="wnat_t")
                        rc = rcp_pool.tile([128, 2, zn, 1], F32, name="rc_t")
                        for m in range(2):
                            pending.append(weights_chunk(q, j, m, ctxu, wnat, rc))
                        for zz in range(zn):
                            pending.append(wtsT_chunk(q, j, zz, wnat, wtsT_sb))
                    for zz in range(zn):
                        out_sb = outsb_pool.tile([128, d], F32, name="out_sb")
                        for h2 in range(d // 512):
                            pending.append(oproj_chunk(q, zz, h2, wtsT_sb, out_sb))
                while pending:
                    pending.pop(0)()
    return nc


_CACHE = {}


def _get_program():
    if "nc" not in _CACHE:
        nc = bacc.Bacc("TRN2")
        build_program(nc)
        nc.compile()
        _CACHE["nc"] = nc
    return _CACHE["nc"]


def kernel(query, key, value, mask, Wq, bq, Wk, bk, Wv, bv, Wo, bo, trace=False):
    f32 = lambda a: np.ascontiguousarray(a, dtype=np.float32)
    bf16 = lambda a: np.ascontiguousarray(np.asarray(a, dtype=np.float32), dtype=ml_dtypes.bfloat16)
    query, key, value, mask = f32(query), f32(key), f32(value), f32(mask)
    Wq, bq, Wk, bk, Wv, bv, Wo, bo = map(f32, (Wq, bq, Wk, bk, Wv, bv, Wo, bo))

    xT = {}
    for b in range(B):
        xT[("q", b)] = bf16(query[b].T)
        xT[("k", b)] = bf16(key[b].T)
        xT[("v", b)] = bf16(value[b].T)

    in_maps = []
    for c in range(NCORES):
        b, g = c // 2, c % 2
        cols = slice(g * HDK, (g + 1) * HDK)
        in_maps.append({
            "xqT": xT[("q", b)], "xkT": xT[("k", b)], "xvT": xT[("v", b)],
            "wq": bf16(Wq[:, cols]), "wk": bf16(Wk[:, cols]), "wv": bf16(Wv[:, cols]),
            "bq": bq[cols].reshape(HDK // 128, 128, 1),
            "bk": bk[cols].reshape(HDK // 128, 128, 1),
            "wo": bf16(Wo[cols, :]),
            "msk": mask[b, 0, 0].reshape(S // 128, 128, 1),
        })

    nc = _get_program()
    res = run_bass_kernel_spmd(
        nc, in_maps, core_ids=list(range(NCORES)), trace=trace
    )

    vo_row = (bv @ Wo + bo).astype(np.float32)  # [D]
    output = np.empty((B, S, D), np.float32)
    weights = np.empty((B, S, H * DV), np.float32)
    for b in range(B):
        output[b] = (res.results[2 * b]["out_p"] + res.results[2 * b + 1]["out_p"]
                     + vo_row)
        weights[b, :, 0:HDK] = (
            np.asarray(res.results[2 * b]["wts_p"], dtype=np.float32) + bv[0:HDK]
        )
        weights[b, :, HDK:] = (
            np.asarray(res.results[2 * b + 1]["wts_p"], dtype=np.float32) + bv[HDK:]
        )
    if trace:
        _CACHE["last_exec_time_ns"] = res.exec_time_ns
        _CACHE["last_res"] = res
    return output, weights
